# revision 48
# baseline (speedup 1.0000x reference)
"""DCGRU cell Trainium2 kernel (8-core data-parallel over batch).

Math (per core, B_loc=4):
  gconv(x, W, b) = sum_m (A_m x) @ W_m + b,  A = [I, S0, 2S0^2-I, S1, 2S1^2-I]
  value = sigmoid(gconv1(concat(inp, hx)));  r, u = split(value)
  c = tanh(gconv2(concat(inp, r*hx)));  new = u*hx + (1-u)*c

Device design (vs the fp16 baseline: 383us -> ~245us):
  * Diffusion runs as out[n_tile,(b,c)] += ST[mc-pair, n-tile]^T @ x[mc-pair]
    in fp8(e4m3) DoubleRow perf mode (2 K-tiles per matmul, 2x+ PE
    throughput), fp32 PSUM.  Supports are host-scaled by SS=2048 into fp8
    range; inverse scales fold into the PSUM-evacuating vector ops.  Both
    supports are SBUF-resident in fp8 (32KB/partition each) - no reloads.
  * Hop-1 results evacuate twice: fp16 on DVE (dense-stage input, stored as
    2*y so the Chebyshev combine is one op; W rows for m in {1,3} halved on
    host) and fp8*SX1 on ScalarE (rhs of the hop-2 S-apply).
  * The gconv1 dense stage is split in halves around the S1 passes: m in
    {0,1,2} (I + S0 chain) right after the S0 passes -> partial pd, m in
    {3,4} after the S1 passes.  This fills the window where the st1 DMA is
    still streaming in.  PSUM->SBUF transpose evacs split DVE/ScalarE.
  * x0^T's hx rows are built on-device by PE transposes (only the 3 tiny
    inp/ones rows ship from host); dense m=0 reads the resident x0T
    directly.  hx for the final combine is a view of x0's u-columns.
  * DMA: partition-major DRAM layouts (2KB+ contiguous lines), ~0.3-0.7MB
    descriptors split across the sync/gpsimd/scalar queues in consumption
    order (descriptor issue costs ~1.3us of the issuing engine; queues run
    ~50-120GB/s each).  Output stores round-robin across queues, fp16.
"""

import sys

if "/opt/trn_rl_repo" not in sys.path:
    sys.path.insert(0, "/opt/trn_rl_repo")

import ml_dtypes
import numpy as np

import concourse.bass as bass
import concourse.mybir as mybir
import concourse.tile as tile
from concourse.bass_utils import run_bass_kernel_spmd
from concourse.masks import make_identity

F8 = mybir.dt.float8e4
F16 = mybir.dt.float16
F32 = mybir.dt.float32
DR = mybir.MatmulPerfMode.DoubleRow

N = 2048          # nodes
U = 64            # units
D = 2             # input dim
C = D + U         # 66 channels after concat
M = 5             # diffusion matrices
B = 32            # global batch
NCORES = 8
BL = B // NCORES  # 4 per-core batch
NT = N // 128     # 16 node tiles
NQ = 16           # dense-stage chunks

SS = 2048.0       # support fp8 scale (entries in [0,1) after scaling)
SX1 = 64.0        # hop-1 fp8 copy scale (y1 entries ~0.015 std -> ~1 std)
SXD = 16.0        # dense-stage fp8 x^T scale
SWD = 16.0        # dense-stage fp8 weight scale


def _split_drain_waits(nc):
    """This walrus build accepts only one sync-wait per instruction on several
    ISA formats; hoist extra waits onto single-wait NoOps placed before (same
    engine, so program order preserves the semantics)."""
    cnt = 0
    for f in nc.m.functions:
        for blk in f.blocks:
            new = []
            for inst in blk.instructions:
                si = inst.sync_info
                if si is not None and len(si.on_wait) > 1:
                    waits = list(si.on_wait)
                    for w in waits[:-1]:
                        cnt += 1
                        n = mybir.InstNoOp(name=f"I-dsplit-{cnt}", ins=[], outs=[])
                        n.engine = inst.engine
                        n.sync_info = mybir.SyncInfo(on_wait=[w], on_update=[])
                        new.append(n)
                    inst.sync_info = mybir.SyncInfo(
                        on_wait=[waits[-1]], on_update=list(si.on_update)
                    )
                new.append(inst)
            blk.instructions = new
    return cnt


def _build_nc():
    nc = bass.Bass()

    # DRAM parameters (host-prepped layouts)
    d_st0 = nc.dram_tensor("st0", [128, NT, N], F8, kind="ExternalInput")
    d_st1 = nc.dram_tensor("st1", [128, NT, NT, 128], F8, kind="ExternalInput")
    d_x0 = nc.dram_tensor("x0h", [128, NT, BL, C], F16, kind="ExternalInput")
    d_x08 = nc.dram_tensor("x08", [128, NT, BL, C], F8, kind="ExternalInput")
    d_x0ti = nc.dram_tensor("x0ti", [3, BL, N], F16, kind="ExternalInput")
    d_wru = nc.dram_tensor("wru", [C + 1, M, 2 * U], F16, kind="ExternalInput")
    d_wc = nc.dram_tensor("wc", [C + 1, M, U], F16, kind="ExternalInput")
    d_w8ru = nc.dram_tensor("w8ru", [C, 2, 2, 2 * U], F8, kind="ExternalInput")
    d_w8c = nc.dram_tensor("w8c", [C, 2, 2, U], F8, kind="ExternalInput")
    d_out = nc.dram_tensor("out", [NT, 128, BL, U], F16, kind="ExternalOutput")

    with tile.TileContext(nc) as tc:
        with (
            tc.tile_pool(name="const", bufs=1) as const_pool,
            tc.tile_pool(name="xbufs", bufs=1) as xbufs,
            tc.tile_pool(name="xtq", bufs=3) as xtq_pool,
            tc.tile_pool(name="cbuf", bufs=2) as c_pool,
            tc.tile_pool(name="pre", bufs=2) as pre_pool,
            tc.tile_pool(name="rp", bufs=2) as r_pool,
            tc.tile_pool(name="dps", bufs=2, space="PSUM") as diff_ps,
            tc.tile_pool(name="tps", bufs=2, space="PSUM") as tr_ps,
            tc.tile_pool(name="nps", bufs=2, space="PSUM") as dense_ps,
        ):
            # ---- resident constants ----
            # All DRAM layouts are partition-major (contiguous 2KB+ lines per
            # partition).  Descriptor issue costs ~1.3us of the issuing
            # engine, and a queue's descriptors transfer mostly serially, so
            # use ~350-700KB descriptors interleaved across the three
            # DMA-capable queues (sync/gpsimd/scalar; scalar is idle early)
            # in consumption order: x08+st0 gate the first matmul, x0 is
            # needed at hop-2 (~+20us), st1 at pass 3 (~+35us, consumed in
            # nt order), x0t at dense-0, hxf at dense-1.
            x08 = xbufs.tile([128, NT, BL, C], F8, tag="x08")
            st0 = const_pool.tile([128, NT, N], F8)
            st1 = const_pool.tile([128, NT, NT, 128], F8)
            x0 = xbufs.tile([128, NT, BL, C], F16, tag="x0")
            x0t = xbufs.tile([C + 1, BL, N], F16, tag="x0t")
            # Observed queue rates: scalar ~120GB/s, sync ~55, gpsimd ~25-50.
            # Split every tensor across queues proportional to speed, ordered
            # by consumption deadline: x08+st0 (first matmul), x0 (hop-2),
            # x0t+w (half-dense-A), st1 (S1 passes, consumed in nt order).
            engs3 = [nc.scalar, nc.sync, nc.gpsimd]
            shares = [(0, 6), (6, 12), (12, 16)]
            for eng, (lo, hi) in zip(engs3, shares):
                eng.dma_start(out=x08[:, lo:hi], in_=d_x08[:, lo:hi])
            for eng, (lo, hi) in zip(engs3, shares):
                mid = (lo + hi) // 2
                eng.dma_start(out=st0[:, lo:mid, :], in_=d_st0[:, lo:mid, :])
                eng.dma_start(out=st0[:, mid:hi, :], in_=d_st0[:, mid:hi, :])
            for eng, (lo, hi) in zip(engs3, shares):
                eng.dma_start(out=x0[:, lo:hi], in_=d_x0[:, lo:hi])
            ident = const_pool.tile([128, 128], F16)
            make_identity(nc, ident)
            wall = const_pool.tile([C + 1, M, 3 * U], F16)
            wru = wall[:, :, 0:2 * U]
            wc = wall[:, :, 2 * U:3 * U]
            nc.sync.dma_start(out=wru, in_=d_wru[:, :, :])
            nc.sync.dma_start(out=wc, in_=d_wc[:, :, :])
            w8ru = const_pool.tile([C, 2, 2, 2 * U], F8)
            w8c = const_pool.tile([C, 2, 2, U], F8)
            nc.sync.dma_start(out=w8ru, in_=d_w8ru[:])
            nc.sync.dma_start(out=w8c, in_=d_w8c[:])
            nc.scalar.dma_start(out=x0t[U:C + 1], in_=d_x0ti[:])
            # st1 pieces in nt (consumption) order, round-robin by speed
            st1_engs = [nc.scalar, nc.sync, nc.gpsimd]
            for i in range(8):
                eng = st1_engs[i % 3]
                eng.dma_start(out=st1[:, 2 * i:2 * i + 2],
                              in_=d_st1[:, 2 * i:2 * i + 2])

            # diffusion outputs (m=1..4), full 66 channels, fp16
            xh = [xbufs.tile([128, NT, BL, C], F16, tag=f"xh{i}", name=f"xh{i}")
                  for i in range(4)]
            # fp8 copies of hop-1 results (rhs of hop-2), one per support
            x18 = [xbufs.tile([128, NT, BL, C], F8, tag=f"x18_{s}", name=f"x18_{s}")
                   for s in range(2)]
            x18g1 = [xbufs.tile([128, NT, BL, U], F8, tag=f"x18g1_{s}",
                                name=f"x18g1_{s}") for s in range(2)]
            # gconv2 state r*hx (u-columns only)
            xhp = xbufs.tile([128, NT, BL, U], F16, tag="xhp")
            xhp8 = xbufs.tile([128, NT, BL, U], F8, tag="xhp8")
            u_sb = xbufs.tile([128, NT, BL, U], F16, tag="u")
            # gconv1 partial dense accumulator (m in {0,1,2})
            pd = xbufs.tile([128, NT, BL, 2 * U], F16, tag="pd")

            def diffusion_pass(gi, s, hop):
                """One S-application; writes xh[2s+hop] (u-cols when gi=1)."""
                nfree = C if gi == 0 else U
                x_first8 = x08 if gi == 0 else xhp8
                dst = xh[2 * s + hop]
                d8buf = x18[s] if gi == 0 else x18g1[s]
                for nt in range(NT):
                    ps = diff_ps.tile([128, BL, nfree], F32, tag="dps")
                    for mc in range(0, NT, 2):
                        if s == 0:
                            lhsT = st0[:, mc:mc + 2, nt * 128:(nt + 1) * 128]
                        else:
                            lhsT = st1[:, nt, mc:mc + 2, :]
                        rhs = x_first8[:, mc:mc + 2] if hop == 0 else d8buf[:, mc:mc + 2]
                        nc.tensor.matmul(
                            ps, lhsT, rhs,
                            start=(mc == 0), stop=(mc == NT - 2),
                            perf_mode=DR,
                        )
                    dst_ap = dst[:, nt, :, :] if gi == 0 else dst[:, nt, :, 0:U]
                    if hop == 0:
                        # xh = 2*y1 (stored scaled; W rows halved on host)
                        nc.vector.tensor_scalar_mul(dst_ap, ps, 2.0 / SS)
                        # fp8 copy = SX1*y1 for the hop-2 S-apply
                        nc.scalar.activation(
                            out=d8buf[:, nt],
                            in_=ps,
                            func=mybir.ActivationFunctionType.Copy,
                            scale=SX1 / SS,
                        )
                    else:
                        # x2' = 2*(S y1); the -x0 (resp -xhp) term of the
                        # Chebyshev recursion is folded into the m0 dense
                        # weights (W0' = W0 - W2 - W4), keeping the fp8 dense
                        # tiles small-valued
                        nc.vector.tensor_scalar_mul(
                            dst_ap, ps, 2.0 / (SS * SX1)
                        )

            def stage_transposes(gi, q, ms):
                """PE transposes + scaled fp8 copies for this chunk's xT."""
                nt = q
                qs = slice(nt * 128, (nt + 1) * 128)
                xtc8 = xtq_pool.tile([C, 4, BL, 128], F8, tag="xt",
                                     name=f"xt_g{gi}_q{q}_{ms[0]}")
                xt1 = None
                if gi == 1:
                    # m=0 tile (f16): xhp^T + inp/ones rows from resident x0t
                    xt1 = pre_pool.tile([C + 1, 1, BL, 128], F16, tag="xt1",
                                        name=f"xt1_q{q}")
                    nc.vector.tensor_copy(xt1[U:C + 1, 0, :, :],
                                          x0t[U:C + 1, :, qs])
                    pst = tr_ps.tile([C, BL, 128], F16, tag="tps", name="pst0")
                    for b in range(BL):
                        nc.tensor.transpose(pst[0:U, b, :], xhp[:, nt, b, :], ident)
                    nc.vector.tensor_copy(xt1[0:U, 0, :, :], pst[0:U])
                if gi == 0 and ms[0] == 1:
                    # build x0t u-rows (hx^T) for this chunk on-device
                    pstx = tr_ps.tile([C, BL, 128], F16, tag="tps", name="pstx")
                    for b in range(BL):
                        nc.tensor.transpose(
                            pstx[0:U, b, :], x0[:, nt, b, 0:U], ident
                        )
                    nc.scalar.activation(
                        out=x0t[0:U, :, qs],
                        in_=pstx[0:U],
                        func=mybir.ActivationFunctionType.Copy,
                    )
                for m in ms:
                    srcb = xh[m - 1]
                    pst = tr_ps.tile([C, BL, 128], F16, tag="tps", name="pstm")
                    for b in range(BL):
                        nc.tensor.transpose(pst[:, b, :], srcb[:, nt, b, :], ident)
                    # scaled fp8 evacs, split between DVE and ScalarE
                    if m % 2 == 1:
                        nc.vector.tensor_scalar_mul(xtc8[0:C, m - 1], pst, SXD)
                    else:
                        nc.scalar.activation(
                            out=xtc8[0:C, m - 1],
                            in_=pst,
                            func=mybir.ActivationFunctionType.Copy,
                            scale=SXD,
                        )
                return (xt1, xtc8)

            def dense_half_a(q, xt):
                """gconv1 partial: m0' (f16) + fp8-DR pair (m1,m2) -> pd."""
                _, x8 = xt
                nt = q
                qs = slice(nt * 128, (nt + 1) * 128)
                t = dense_ps.tile([128, 2, BL, 2 * U], F32, tag="nps")
                ps1, ps2 = t[:, 0], t[:, 1]
                for b in range(BL):
                    nc.tensor.matmul(
                        ps1[:, b, :], x0t[0:C + 1, b, qs], wru[0:C + 1, 0, :],
                        start=True, stop=True,
                    )
                    nc.tensor.matmul(
                        ps2[:, b, :], x8[0:C, 0:2, b, :], w8ru[0:C, 0],
                        start=True, stop=True, perf_mode=DR,
                    )
                nc.vector.tensor_scalar_mul(pd[:, nt], ps2, 1.0 / (SXD * SWD))
                nc.vector.tensor_add(pd[:, nt], pd[:, nt], ps1)

            def dense_half_b(q, xt):
                """gconv1 rest: fp8-DR pair (m3,m4) + pd -> sigmoids, xhp."""
                _, x8 = xt
                nt = q
                t = dense_ps.tile([128, 2, BL, 2 * U], F32, tag="nps")
                ps2 = t[:, 1]
                for b in range(BL):
                    nc.tensor.matmul(
                        ps2[:, b, :], x8[0:C, 2:4, b, :], w8ru[0:C, 1],
                        start=True, stop=True, perf_mode=DR,
                    )
                pre = pre_pool.tile([128, BL, 2 * U], F16, tag="pre")
                nc.vector.scalar_tensor_tensor(
                    out=pre, in0=ps2, scalar=1.0 / (SXD * SWD), in1=pd[:, nt],
                    op0=mybir.AluOpType.mult, op1=mybir.AluOpType.add,
                )
                rt = r_pool.tile([128, BL, U], F16, tag="rt")
                nc.scalar.activation(
                    out=rt, in_=pre[:, :, 0:U],
                    func=mybir.ActivationFunctionType.Sigmoid,
                )
                nc.scalar.activation(
                    out=u_sb[:, nt, :, :], in_=pre[:, :, U:2 * U],
                    func=mybir.ActivationFunctionType.Sigmoid,
                )
                nc.vector.tensor_mul(xhp[:, nt, :, :], rt, x0[:, nt, :, 0:U])
                nc.scalar.activation(
                    out=xhp8[:, nt], in_=xhp[:, nt],
                    func=mybir.ActivationFunctionType.Copy, scale=1.0,
                )

            def stage_dense1(q, xt):
                """gconv2 dense: m0' (f16) + two fp8-DR pairs, then combine."""
                xt1, x8 = xt
                nt = q
                t = dense_ps.tile([128, 2, BL, 2 * U], F32, tag="nps")
                for b in range(BL):
                    off = (b % 2) * U
                    p1 = t[:, 0, b // 2, off:off + U]
                    nc.tensor.matmul(
                        p1, xt1[0:C + 1, 0, b, :], wc[0:C + 1, 0, :],
                        start=True, stop=True,
                    )
                    p2 = t[:, 1, b // 2, off:off + U]
                    nc.tensor.matmul(
                        p2, x8[0:C, 0:2, b, :], w8c[0:C, 0],
                        start=True, stop=False, perf_mode=DR,
                    )
                    nc.tensor.matmul(
                        p2, x8[0:C, 2:4, b, :], w8c[0:C, 1],
                        start=False, stop=True, perf_mode=DR,
                    )
                pre2 = pre_pool.tile([128, BL, U], F16, tag="prez")
                nc.vector.tensor_scalar_mul(pre2, t[:, 1, 0:2, :],
                                            1.0 / (SXD * SWD))
                nc.vector.tensor_add(pre2, pre2, t[:, 0, 0:2, :])
                cpair = c_pool.tile([128, 2, BL, U], F16, tag="cb")
                cb = cpair[:, 0]
                tmp = cpair[:, 1]
                nc.scalar.activation(
                    out=cb, in_=pre2, func=mybir.ActivationFunctionType.Tanh
                )
                # new = c + u*(hx - c); hx is the u-column block of x0
                nc.vector.tensor_sub(tmp, x0[:, nt, :, 0:U], cb)
                nc.vector.tensor_mul(tmp, u_sb[:, nt, :, :], tmp)
                nc.vector.tensor_add(tmp, tmp, cb)
                oeng = [nc.sync, nc.scalar, nc.gpsimd][nt % 3]
                oeng.dma_start(out=d_out[nt], in_=tmp)

            def pipeline(ms, dense_fn, gi):
                prev = stage_transposes(gi, 0, ms)
                for q in range(1, NQ):
                    cur = stage_transposes(gi, q, ms)
                    dense_fn(q - 1, prev)
                    prev = cur
                dense_fn(NQ - 1, prev)

            # gconv1: S0 passes, then the S0-half of the dense stage (covers
            # the st1 DMA), then S1 passes and the rest of the dense stage
            diffusion_pass(0, 0, 0)
            diffusion_pass(0, 0, 1)
            pipeline((1, 2), dense_half_a, 0)
            diffusion_pass(0, 1, 0)
            diffusion_pass(0, 1, 1)
            pipeline((3, 4), dense_half_b, 0)
            # gconv2
            diffusion_pass(1, 0, 0)
            diffusion_pass(1, 0, 1)
            diffusion_pass(1, 1, 0)
            diffusion_pass(1, 1, 1)
            pipeline((1, 2, 3, 4), stage_dense1, 1)

    _split_drain_waits(nc)
    return nc


_NC_CACHE = None


def _get_nc():
    global _NC_CACHE
    if _NC_CACHE is None:
        _NC_CACHE = _build_nc()
    return _NC_CACHE


def _prep_host(inputs, hx, support0, support1, W_ru, b_ru, W_c, b_c):
    f16 = np.float16
    f8 = ml_dtypes.float8_e4m3
    inp = inputs.reshape(B, N, D).astype(np.float32)
    hx3 = hx.reshape(B, N, U).astype(np.float32)
    x0_full = np.concatenate([hx3, inp], axis=2)  # [B, N, C] fp32, u-first

    # partition-major: st0[p, mc, col] = S0^T[mc*128+p, col]
    st0 = np.ascontiguousarray(
        (support0.T.reshape(NT, 128, N).transpose(1, 0, 2)) * SS
    ).astype(f8)
    # st1[p, nt, mc, col] = S1^T[mc*128+p, nt*128+col]
    st1 = np.ascontiguousarray(
        (support1.T.reshape(NT, 128, NT, 128).transpose(1, 2, 0, 3)) * SS
    ).astype(f8)

    def prep_w(W, bvec, osz):
        w = W.reshape(C, M, osz).astype(np.float32)
        w = np.concatenate([w[D:], w[:D]], axis=0).copy()  # u-first rows
        w[:, 1, :] *= 0.5
        w[:, 3, :] *= 0.5
        wf = np.zeros((C + 1, M, osz), np.float32)
        wf[:C] = w
        wf[C, 0, :] = bvec
        # fold the Chebyshev -x0 terms (m=2,4) into m0: the diffusion stores
        # x2' = x2 + x0, so the dense m0 weights absorb -W2 - W4
        wf[:C, 0] -= w[:, 2] + w[:, 4]
        # fp8 pair weights for the DoubleRow dense matmuls
        w8 = np.stack([w[:, 1:3], w[:, 3:5]], axis=1)  # [C, 2, 2, osz]
        w8 = (w8.transpose(0, 1, 2, 3) * SWD).astype(ml_dtypes.float8_e4m3)
        return wf.astype(f16), np.ascontiguousarray(w8)

    wru, w8ru = prep_w(W_ru, b_ru, 2 * U)
    wcc, w8c = prep_w(W_c, b_c, U)

    in_maps = []
    for c in range(NCORES):
        cs = slice(c * BL, (c + 1) * BL)
        x0c = x0_full[cs]                                   # [BL, N, C]
        x0h32 = np.ascontiguousarray(
            x0c.transpose(1, 0, 2).reshape(NT, 128, BL, C).transpose(1, 0, 2, 3)
        )                                                    # [128, NT, BL, C]
        x0ti = np.concatenate(
            [x0c.transpose(2, 0, 1)[U:], np.ones((1, BL, N), np.float32)], axis=0
        ).astype(f16)                                        # [3, BL, N]
        in_maps.append(
            {
                "st0": st0,
                "st1": st1,
                "x0h": np.ascontiguousarray(x0h32.astype(f16)),
                "x08": np.ascontiguousarray(x0h32.astype(f8)),
                "x0ti": np.ascontiguousarray(x0ti),
                "wru": wru,
                "wc": wcc,
                "w8ru": w8ru,
                "w8c": w8c,
            }
        )
    return in_maps


def kernel(inputs, hx, support0, support1, W_ru, b_ru, W_c, b_c, _trace=False,
           _tmpdir=None):
    nc = _get_nc()
    in_maps = _prep_host(
        inputs, hx, support0, support1, W_ru, b_ru, W_c, b_c
    )
    res = run_bass_kernel_spmd(
        nc, in_maps, core_ids=list(range(NCORES)), trace=_trace, tmpdir=_tmpdir
    )
    out = np.empty((B, N * U), np.float32)
    for c in range(NCORES):
        od = res.results[c]["out"].astype(np.float32)  # [NT, 128, BL, U] f16
        out[c * BL:(c + 1) * BL] = (
            od.transpose(2, 0, 1, 3).reshape(BL, N * U)
        )
    kernel._last_result = res
    return out


# revision 49
# speedup vs baseline: 1.0705x; 1.0705x over previous
"""DCGRU cell Trainium2 kernel (8-core data-parallel over batch).

Math (per core, B_loc=4):
  gconv(x, W, b) = sum_m (A_m x) @ W_m + b,  A = [I, S0, 2S0^2-I, S1, 2S1^2-I]
  value = sigmoid(gconv1(concat(inp, hx)));  r, u = split(value)
  c = tanh(gconv2(concat(inp, r*hx)));  new = u*hx + (1-u)*c

Device design (vs the fp16 baseline: 383us -> ~245us):
  * Diffusion runs as out[n_tile,(b,c)] += ST[mc-pair, n-tile]^T @ x[mc-pair]
    in fp8(e4m3) DoubleRow perf mode (2 K-tiles per matmul, 2x+ PE
    throughput), fp32 PSUM.  Supports are host-scaled by SS=2048 into fp8
    range; inverse scales fold into the PSUM-evacuating vector ops.  Both
    supports are SBUF-resident in fp8 (32KB/partition each) - no reloads.
  * Hop-1 results evacuate twice: fp16 on DVE (dense-stage input, stored as
    2*y so the Chebyshev combine is one op; W rows for m in {1,3} halved on
    host) and fp8*SX1 on ScalarE (rhs of the hop-2 S-apply).
  * The gconv1 dense stage is split in halves around the S1 passes: m in
    {0,1,2} (I + S0 chain) right after the S0 passes -> partial pd, m in
    {3,4} after the S1 passes.  This fills the window where the st1 DMA is
    still streaming in.  PSUM->SBUF transpose evacs split DVE/ScalarE.
  * x0^T's hx rows are built on-device by PE transposes (only the 3 tiny
    inp/ones rows ship from host); dense m=0 reads the resident x0T
    directly.  hx for the final combine is a view of x0's u-columns.
  * DMA: partition-major DRAM layouts (2KB+ contiguous lines), ~0.3-0.7MB
    descriptors split across the sync/gpsimd/scalar queues in consumption
    order (descriptor issue costs ~1.3us of the issuing engine; queues run
    ~50-120GB/s each).  Output stores round-robin across queues, fp16.
"""

import sys

if "/opt/trn_rl_repo" not in sys.path:
    sys.path.insert(0, "/opt/trn_rl_repo")

import ml_dtypes
import numpy as np

import concourse.bass as bass
import concourse.mybir as mybir
import concourse.tile as tile
from concourse.bass_utils import run_bass_kernel_spmd
from concourse.masks import make_identity

F8 = mybir.dt.float8e4
F16 = mybir.dt.float16
F32 = mybir.dt.float32
DR = mybir.MatmulPerfMode.DoubleRow

N = 2048          # nodes
U = 64            # units
D = 2             # input dim
C = D + U         # 66 channels after concat
M = 5             # diffusion matrices
B = 32            # global batch
NCORES = 8
BL = B // NCORES  # 4 per-core batch
NT = N // 128     # 16 node tiles
NQ = 16           # dense-stage chunks

SS = 2048.0       # support fp8 scale (entries in [0,1) after scaling)
SX1 = 64.0        # hop-1 fp8 copy scale (y1 entries ~0.015 std -> ~1 std)
SXD = 16.0        # dense-stage fp8 x^T scale
SWD = 16.0        # dense-stage fp8 weight scale


def _split_drain_waits(nc):
    """This walrus build accepts only one sync-wait per instruction on several
    ISA formats; hoist extra waits onto single-wait NoOps placed before (same
    engine, so program order preserves the semantics)."""
    cnt = 0
    for f in nc.m.functions:
        for blk in f.blocks:
            new = []
            for inst in blk.instructions:
                si = inst.sync_info
                if si is not None and len(si.on_wait) > 1:
                    waits = list(si.on_wait)
                    for w in waits[:-1]:
                        cnt += 1
                        n = mybir.InstNoOp(name=f"I-dsplit-{cnt}", ins=[], outs=[])
                        n.engine = inst.engine
                        n.sync_info = mybir.SyncInfo(on_wait=[w], on_update=[])
                        new.append(n)
                    inst.sync_info = mybir.SyncInfo(
                        on_wait=[waits[-1]], on_update=list(si.on_update)
                    )
                new.append(inst)
            blk.instructions = new
    return cnt


def _build_nc():
    nc = bass.Bass()

    # DRAM parameters (host-prepped layouts)
    d_st0 = nc.dram_tensor("st0", [128, NT, N], F8, kind="ExternalInput")
    d_st1 = nc.dram_tensor("st1", [128, NT, NT, 128], F8, kind="ExternalInput")
    d_x0 = nc.dram_tensor("x0h", [128, NT, BL, C], F16, kind="ExternalInput")
    d_x08 = nc.dram_tensor("x08", [128, NT, BL, C], F8, kind="ExternalInput")
    d_x0ti = nc.dram_tensor("x0ti", [3, BL, N], F16, kind="ExternalInput")
    d_wru = nc.dram_tensor("wru", [C + 1, M, 2 * U], F16, kind="ExternalInput")
    d_wc = nc.dram_tensor("wc", [C + 1, M, U], F16, kind="ExternalInput")
    d_w8ru = nc.dram_tensor("w8ru", [C, 2, 2, 2 * U], F8, kind="ExternalInput")
    d_w8c = nc.dram_tensor("w8c", [C, 2, 2, U], F8, kind="ExternalInput")
    d_out = nc.dram_tensor("out", [NT, 128, BL, U], F16, kind="ExternalOutput")

    with tile.TileContext(nc) as tc:
        with (
            tc.tile_pool(name="const", bufs=1) as const_pool,
            tc.tile_pool(name="xbufs", bufs=1) as xbufs,
            tc.tile_pool(name="xtq", bufs=3) as xtq_pool,
            tc.tile_pool(name="cbuf", bufs=2) as c_pool,
            tc.tile_pool(name="pre", bufs=2) as pre_pool,
            tc.tile_pool(name="rp", bufs=2) as r_pool,
            tc.tile_pool(name="dps", bufs=3, space="PSUM") as diff_ps,
            tc.tile_pool(name="tps", bufs=3, space="PSUM") as tr_ps,
            tc.tile_pool(name="nps", bufs=2, space="PSUM") as dense_ps,
        ):
            # ---- resident constants ----
            # All DRAM layouts are partition-major (contiguous 2KB+ lines per
            # partition).  Descriptor issue costs ~1.3us of the issuing
            # engine, and a queue's descriptors transfer mostly serially, so
            # use ~350-700KB descriptors interleaved across the three
            # DMA-capable queues (sync/gpsimd/scalar; scalar is idle early)
            # in consumption order: x08+st0 gate the first matmul, x0 is
            # needed at hop-2 (~+20us), st1 at pass 3 (~+35us, consumed in
            # nt order), x0t at dense-0, hxf at dense-1.
            x08 = xbufs.tile([128, NT, BL, C], F8, tag="x08")
            st0 = const_pool.tile([128, NT, N], F8)
            st1 = const_pool.tile([128, NT, NT, 128], F8)
            x0 = xbufs.tile([128, NT, BL, C], F16, tag="x0")
            x0t = xbufs.tile([C + 1, BL, N], F16, tag="x0t")
            # Observed queue rates: scalar ~120GB/s, sync ~55, gpsimd ~25-50.
            # Split every tensor across queues proportional to speed, ordered
            # by consumption deadline: x08+st0 (first matmul), x0 (hop-2),
            # x0t+w (half-dense-A), st1 (S1 passes, consumed in nt order).
            engs3 = [nc.scalar, nc.sync, nc.gpsimd]
            shares = [(0, 6), (6, 12), (12, 16)]
            for eng, (lo, hi) in zip(engs3, shares):
                eng.dma_start(out=x08[:, lo:hi], in_=d_x08[:, lo:hi])
            for eng, (lo, hi) in zip(engs3, shares):
                mid = (lo + hi) // 2
                eng.dma_start(out=st0[:, lo:mid, :], in_=d_st0[:, lo:mid, :])
                eng.dma_start(out=st0[:, mid:hi, :], in_=d_st0[:, mid:hi, :])
            for eng, (lo, hi) in zip(engs3, shares):
                eng.dma_start(out=x0[:, lo:hi], in_=d_x0[:, lo:hi])
            ident = const_pool.tile([128, 128], F16)
            make_identity(nc, ident)
            wall = const_pool.tile([C + 1, M, 3 * U], F16)
            wru = wall[:, :, 0:2 * U]
            wc = wall[:, :, 2 * U:3 * U]
            nc.sync.dma_start(out=wru, in_=d_wru[:, :, :])
            nc.sync.dma_start(out=wc, in_=d_wc[:, :, :])
            w8ru = const_pool.tile([C, 2, 2, 2 * U], F8)
            w8c = const_pool.tile([C, 2, 2, U], F8)
            nc.sync.dma_start(out=w8ru, in_=d_w8ru[:])
            nc.sync.dma_start(out=w8c, in_=d_w8c[:])
            nc.scalar.dma_start(out=x0t[U:C + 1], in_=d_x0ti[:])
            # st1 pieces in nt (consumption) order, round-robin by speed
            st1_engs = [nc.scalar, nc.sync, nc.gpsimd]
            for i in range(8):
                eng = st1_engs[i % 3]
                eng.dma_start(out=st1[:, 2 * i:2 * i + 2],
                              in_=d_st1[:, 2 * i:2 * i + 2])

            # diffusion outputs (m=1..4), full 66 channels, fp16
            xh = [xbufs.tile([128, NT, BL, C], F16, tag=f"xh{i}", name=f"xh{i}")
                  for i in range(4)]
            # fp8 copies of hop-1 results (rhs of hop-2), one per support
            x18 = [xbufs.tile([128, NT, BL, C], F8, tag=f"x18_{s}", name=f"x18_{s}")
                   for s in range(2)]
            x18g1 = [xbufs.tile([128, NT, BL, U], F8, tag=f"x18g1_{s}",
                                name=f"x18g1_{s}") for s in range(2)]
            # gconv2 state r*hx (u-columns only)
            xhp = xbufs.tile([128, NT, BL, U], F16, tag="xhp")
            xhp8 = xbufs.tile([128, NT, BL, U], F8, tag="xhp8")
            u_sb = xbufs.tile([128, NT, BL, U], F16, tag="u")
            # gconv1 partial dense accumulator (m in {0,1,2})
            pd = xbufs.tile([128, NT, BL, 2 * U], F16, tag="pd")

            def diffusion_pass(gi, s, hop):
                """One S-application; writes xh[2s+hop] (u-cols when gi=1)."""
                nfree = C if gi == 0 else U
                x_first8 = x08 if gi == 0 else xhp8
                dst = xh[2 * s + hop]
                d8buf = x18[s] if gi == 0 else x18g1[s]
                for nt in range(NT):
                    ps = diff_ps.tile([128, BL, nfree], F32, tag="dps")
                    for mc in range(0, NT, 2):
                        if s == 0:
                            lhsT = st0[:, mc:mc + 2, nt * 128:(nt + 1) * 128]
                        else:
                            lhsT = st1[:, nt, mc:mc + 2, :]
                        rhs = x_first8[:, mc:mc + 2] if hop == 0 else d8buf[:, mc:mc + 2]
                        nc.tensor.matmul(
                            ps, lhsT, rhs,
                            start=(mc == 0), stop=(mc == NT - 2),
                            perf_mode=DR,
                        )
                    dst_ap = dst[:, nt, :, :] if gi == 0 else dst[:, nt, :, 0:U]
                    if hop == 0:
                        # xh = 2*y1 (stored scaled; W rows halved on host)
                        nc.vector.tensor_scalar_mul(dst_ap, ps, 2.0 / SS)
                        # fp8 copy = SX1*y1 for the hop-2 S-apply
                        nc.scalar.activation(
                            out=d8buf[:, nt],
                            in_=ps,
                            func=mybir.ActivationFunctionType.Copy,
                            scale=SX1 / SS,
                        )
                    else:
                        # x2' = 2*(S y1); the -x0 (resp -xhp) term of the
                        # Chebyshev recursion is folded into the m0 dense
                        # weights (W0' = W0 - W2 - W4), keeping the fp8 dense
                        # tiles small-valued
                        nc.vector.tensor_scalar_mul(
                            dst_ap, ps, 2.0 / (SS * SX1)
                        )

            def stage_transposes(gi, q, ms):
                """PE transposes + scaled fp8 copies for this chunk's xT."""
                nt = q
                qs = slice(nt * 128, (nt + 1) * 128)
                xtc8 = xtq_pool.tile([C, 4, BL, 128], F8, tag="xt",
                                     name=f"xt_g{gi}_q{q}_{ms[0]}")
                xt1 = None
                if gi == 1:
                    # m=0 tile (f16): xhp^T + inp/ones rows from resident x0t
                    xt1 = pre_pool.tile([C + 1, 1, BL, 128], F16, tag="xt1",
                                        name=f"xt1_q{q}")
                    nc.vector.tensor_copy(xt1[U:C + 1, 0, :, :],
                                          x0t[U:C + 1, :, qs])
                    pst = tr_ps.tile([C, BL, 128], F16, tag="tps", name="pst0")
                    for b in range(BL):
                        nc.tensor.transpose(pst[0:U, b, :], xhp[:, nt, b, :], ident)
                    nc.vector.tensor_copy(xt1[0:U, 0, :, :], pst[0:U])
                if gi == 0 and ms[0] == 1:
                    # build x0t u-rows (hx^T) for this chunk on-device
                    pstx = tr_ps.tile([C, BL, 128], F16, tag="tps", name="pstx")
                    for b in range(BL):
                        nc.tensor.transpose(
                            pstx[0:U, b, :], x0[:, nt, b, 0:U], ident
                        )
                    nc.scalar.activation(
                        out=x0t[0:U, :, qs],
                        in_=pstx[0:U],
                        func=mybir.ActivationFunctionType.Copy,
                    )
                for m in ms:
                    srcb = xh[m - 1]
                    pst = tr_ps.tile([C, BL, 128], F16, tag="tps", name="pstm")
                    for b in range(BL):
                        nc.tensor.transpose(pst[:, b, :], srcb[:, nt, b, :], ident)
                    # scaled fp8 evacs, split between DVE and ScalarE
                    if m % 2 == 1:
                        nc.vector.tensor_scalar_mul(xtc8[0:C, m - 1], pst, SXD)
                    else:
                        nc.scalar.activation(
                            out=xtc8[0:C, m - 1],
                            in_=pst,
                            func=mybir.ActivationFunctionType.Copy,
                            scale=SXD,
                        )
                return (xt1, xtc8)

            def dense_half_a(q, xt):
                """gconv1 partial: m0' (f16) + fp8-DR pair (m1,m2) -> pd."""
                _, x8 = xt
                nt = q
                qs = slice(nt * 128, (nt + 1) * 128)
                t1 = dense_ps.tile([128, BL, 2 * U], F32, tag="nps")
                for b in range(BL):
                    nc.tensor.matmul(
                        t1[:, b, :], x0t[0:C + 1, b, qs], wru[0:C + 1, 0, :],
                        start=True, stop=True,
                    )
                nc.vector.tensor_copy(pd[:, nt], t1)
                t2 = dense_ps.tile([128, BL, 2 * U], F32, tag="nps")
                for b in range(BL):
                    nc.tensor.matmul(
                        t2[:, b, :], x8[0:C, 0:2, b, :], w8ru[0:C, 0],
                        start=True, stop=True, perf_mode=DR,
                    )
                nc.vector.scalar_tensor_tensor(
                    out=pd[:, nt], in0=t2, scalar=1.0 / (SXD * SWD),
                    in1=pd[:, nt],
                    op0=mybir.AluOpType.mult, op1=mybir.AluOpType.add,
                )

            def dense_half_b(q, xt):
                """gconv1 rest: fp8-DR pair (m3,m4) + pd -> sigmoids, xhp."""
                _, x8 = xt
                nt = q
                t2 = dense_ps.tile([128, BL, 2 * U], F32, tag="nps")
                for b in range(BL):
                    nc.tensor.matmul(
                        t2[:, b, :], x8[0:C, 2:4, b, :], w8ru[0:C, 1],
                        start=True, stop=True, perf_mode=DR,
                    )
                pre = pre_pool.tile([128, BL, 2 * U], F16, tag="pre")
                nc.vector.scalar_tensor_tensor(
                    out=pre, in0=t2, scalar=1.0 / (SXD * SWD), in1=pd[:, nt],
                    op0=mybir.AluOpType.mult, op1=mybir.AluOpType.add,
                )
                rt = r_pool.tile([128, BL, U], F16, tag="rt")
                nc.scalar.activation(
                    out=rt, in_=pre[:, :, 0:U],
                    func=mybir.ActivationFunctionType.Sigmoid,
                )
                nc.scalar.activation(
                    out=u_sb[:, nt, :, :], in_=pre[:, :, U:2 * U],
                    func=mybir.ActivationFunctionType.Sigmoid,
                )
                nc.vector.tensor_mul(xhp[:, nt, :, :], rt, x0[:, nt, :, 0:U])
                nc.scalar.activation(
                    out=xhp8[:, nt], in_=xhp[:, nt],
                    func=mybir.ActivationFunctionType.Copy, scale=1.0,
                )

            def stage_dense1(q, xt):
                """gconv2 dense: m0' (f16) + two fp8-DR pairs, then combine."""
                xt1, x8 = xt
                nt = q
                t1 = dense_ps.tile([128, BL, 2 * U], F32, tag="nps")
                for b in range(BL):
                    off = (b % 2) * U
                    nc.tensor.matmul(
                        t1[:, b // 2, off:off + U],
                        xt1[0:C + 1, 0, b, :], wc[0:C + 1, 0, :],
                        start=True, stop=True,
                    )
                t2 = dense_ps.tile([128, BL, 2 * U], F32, tag="nps")
                for b in range(BL):
                    off = (b % 2) * U
                    p2 = t2[:, b // 2, off:off + U]
                    nc.tensor.matmul(
                        p2, x8[0:C, 0:2, b, :], w8c[0:C, 0],
                        start=True, stop=False, perf_mode=DR,
                    )
                    nc.tensor.matmul(
                        p2, x8[0:C, 2:4, b, :], w8c[0:C, 1],
                        start=False, stop=True, perf_mode=DR,
                    )
                pre2 = pre_pool.tile([128, BL, U], F16, tag="prez")
                nc.vector.tensor_scalar_mul(pre2, t2[:, 0:2, :],
                                            1.0 / (SXD * SWD))
                nc.vector.tensor_add(pre2, pre2, t1[:, 0:2, :])
                cpair = c_pool.tile([128, 2, BL, U], F16, tag="cb")
                cb = cpair[:, 0]
                tmp = cpair[:, 1]
                nc.scalar.activation(
                    out=cb, in_=pre2, func=mybir.ActivationFunctionType.Tanh
                )
                # new = c + u*(hx - c); hx is the u-column block of x0
                nc.vector.tensor_sub(tmp, x0[:, nt, :, 0:U], cb)
                nc.vector.tensor_mul(tmp, u_sb[:, nt, :, :], tmp)
                nc.vector.tensor_add(tmp, tmp, cb)
                oeng = [nc.sync, nc.scalar, nc.gpsimd][nt % 3]
                oeng.dma_start(out=d_out[nt], in_=tmp)

            def pipeline(ms, dense_fn, gi):
                prev = stage_transposes(gi, 0, ms)
                for q in range(1, NQ):
                    cur = stage_transposes(gi, q, ms)
                    dense_fn(q - 1, prev)
                    prev = cur
                dense_fn(NQ - 1, prev)

            # gconv1: S0 passes, then the S0-half of the dense stage (covers
            # the st1 DMA), then S1 passes and the rest of the dense stage
            diffusion_pass(0, 0, 0)
            diffusion_pass(0, 0, 1)
            pipeline((1, 2), dense_half_a, 0)
            diffusion_pass(0, 1, 0)
            diffusion_pass(0, 1, 1)
            pipeline((3, 4), dense_half_b, 0)
            # gconv2
            diffusion_pass(1, 0, 0)
            diffusion_pass(1, 0, 1)
            diffusion_pass(1, 1, 0)
            diffusion_pass(1, 1, 1)
            pipeline((1, 2, 3, 4), stage_dense1, 1)

    _split_drain_waits(nc)
    return nc


_NC_CACHE = None


def _get_nc():
    global _NC_CACHE
    if _NC_CACHE is None:
        _NC_CACHE = _build_nc()
    return _NC_CACHE


def _prep_host(inputs, hx, support0, support1, W_ru, b_ru, W_c, b_c):
    f16 = np.float16
    f8 = ml_dtypes.float8_e4m3
    inp = inputs.reshape(B, N, D).astype(np.float32)
    hx3 = hx.reshape(B, N, U).astype(np.float32)
    x0_full = np.concatenate([hx3, inp], axis=2)  # [B, N, C] fp32, u-first

    # partition-major: st0[p, mc, col] = S0^T[mc*128+p, col]
    st0 = np.ascontiguousarray(
        (support0.T.reshape(NT, 128, N).transpose(1, 0, 2)) * SS
    ).astype(f8)
    # st1[p, nt, mc, col] = S1^T[mc*128+p, nt*128+col]
    st1 = np.ascontiguousarray(
        (support1.T.reshape(NT, 128, NT, 128).transpose(1, 2, 0, 3)) * SS
    ).astype(f8)

    def prep_w(W, bvec, osz):
        w = W.reshape(C, M, osz).astype(np.float32)
        w = np.concatenate([w[D:], w[:D]], axis=0).copy()  # u-first rows
        w[:, 1, :] *= 0.5
        w[:, 3, :] *= 0.5
        wf = np.zeros((C + 1, M, osz), np.float32)
        wf[:C] = w
        wf[C, 0, :] = bvec
        # fold the Chebyshev -x0 terms (m=2,4) into m0: the diffusion stores
        # x2' = x2 + x0, so the dense m0 weights absorb -W2 - W4
        wf[:C, 0] -= w[:, 2] + w[:, 4]
        # fp8 pair weights for the DoubleRow dense matmuls
        w8 = np.stack([w[:, 1:3], w[:, 3:5]], axis=1)  # [C, 2, 2, osz]
        w8 = (w8.transpose(0, 1, 2, 3) * SWD).astype(ml_dtypes.float8_e4m3)
        return wf.astype(f16), np.ascontiguousarray(w8)

    wru, w8ru = prep_w(W_ru, b_ru, 2 * U)
    wcc, w8c = prep_w(W_c, b_c, U)

    in_maps = []
    for c in range(NCORES):
        cs = slice(c * BL, (c + 1) * BL)
        x0c = x0_full[cs]                                   # [BL, N, C]
        x0h32 = np.ascontiguousarray(
            x0c.transpose(1, 0, 2).reshape(NT, 128, BL, C).transpose(1, 0, 2, 3)
        )                                                    # [128, NT, BL, C]
        x0ti = np.concatenate(
            [x0c.transpose(2, 0, 1)[U:], np.ones((1, BL, N), np.float32)], axis=0
        ).astype(f16)                                        # [3, BL, N]
        in_maps.append(
            {
                "st0": st0,
                "st1": st1,
                "x0h": np.ascontiguousarray(x0h32.astype(f16)),
                "x08": np.ascontiguousarray(x0h32.astype(f8)),
                "x0ti": np.ascontiguousarray(x0ti),
                "wru": wru,
                "wc": wcc,
                "w8ru": w8ru,
                "w8c": w8c,
            }
        )
    return in_maps


def kernel(inputs, hx, support0, support1, W_ru, b_ru, W_c, b_c, _trace=False,
           _tmpdir=None):
    nc = _get_nc()
    in_maps = _prep_host(
        inputs, hx, support0, support1, W_ru, b_ru, W_c, b_c
    )
    res = run_bass_kernel_spmd(
        nc, in_maps, core_ids=list(range(NCORES)), trace=_trace, tmpdir=_tmpdir
    )
    out = np.empty((B, N * U), np.float32)
    for c in range(NCORES):
        od = res.results[c]["out"].astype(np.float32)  # [NT, 128, BL, U] f16
        out[c * BL:(c + 1) * BL] = (
            od.transpose(2, 0, 1, 3).reshape(BL, N * U)
        )
    kernel._last_result = res
    return out


# revision 50
# speedup vs baseline: 1.2448x; 1.1629x over previous
"""DCGRU cell Trainium2 kernel (8-core data-parallel over batch).

Math (per core, B_loc=4):
  gconv(x, W, b) = sum_m (A_m x) @ W_m + b,  A = [I, S0, 2S0^2-I, S1, 2S1^2-I]
  value = sigmoid(gconv1(concat(inp, hx)));  r, u = split(value)
  c = tanh(gconv2(concat(inp, r*hx)));  new = u*hx + (1-u)*c

Device design (vs the fp16 baseline: 383us -> ~245us):
  * Diffusion runs as out[n_tile,(b,c)] += ST[mc-pair, n-tile]^T @ x[mc-pair]
    in fp8(e4m3) DoubleRow perf mode (2 K-tiles per matmul, 2x+ PE
    throughput), fp32 PSUM.  Supports are host-scaled by SS=2048 into fp8
    range; inverse scales fold into the PSUM-evacuating vector ops.  Both
    supports are SBUF-resident in fp8 (32KB/partition each) - no reloads.
  * Hop-1 results evacuate twice: fp16 on DVE (dense-stage input, stored as
    2*y so the Chebyshev combine is one op; W rows for m in {1,3} halved on
    host) and fp8*SX1 on ScalarE (rhs of the hop-2 S-apply).
  * The gconv1 dense stage is split in halves around the S1 passes: m in
    {0,1,2} (I + S0 chain) right after the S0 passes -> partial pd, m in
    {3,4} after the S1 passes.  This fills the window where the st1 DMA is
    still streaming in.  PSUM->SBUF transpose evacs split DVE/ScalarE.
  * x0^T's hx rows are built on-device by PE transposes (only the 3 tiny
    inp/ones rows ship from host); dense m=0 reads the resident x0T
    directly.  hx for the final combine is a view of x0's u-columns.
  * DMA: partition-major DRAM layouts (2KB+ contiguous lines), ~0.3-0.7MB
    descriptors split across the sync/gpsimd/scalar queues in consumption
    order (descriptor issue costs ~1.3us of the issuing engine; queues run
    ~50-120GB/s each).  Output stores round-robin across queues, fp16.
"""

import sys

if "/opt/trn_rl_repo" not in sys.path:
    sys.path.insert(0, "/opt/trn_rl_repo")

import ml_dtypes
import numpy as np

import concourse.bass as bass
import concourse.mybir as mybir
import concourse.tile as tile
from concourse.bass_utils import run_bass_kernel_spmd
from concourse.masks import make_identity

F8 = mybir.dt.float8e4
F16 = mybir.dt.float16
F32 = mybir.dt.float32
DR = mybir.MatmulPerfMode.DoubleRow

N = 2048          # nodes
U = 64            # units
D = 2             # input dim
C = D + U         # 66 channels after concat
M = 5             # diffusion matrices
B = 32            # global batch
NCORES = 8
BL = B // NCORES  # 4 per-core batch
NT = N // 128     # 16 node tiles
NQ = 16           # dense-stage chunks

SS = 2048.0       # support fp8 scale (entries in [0,1) after scaling)
SX1 = 64.0        # hop-1 fp8 copy scale (y1 entries ~0.015 std -> ~1 std)


def _split_drain_waits(nc):
    """This walrus build accepts only one sync-wait per instruction on several
    ISA formats; hoist extra waits onto single-wait NoOps placed before (same
    engine, so program order preserves the semantics)."""
    cnt = 0
    for f in nc.m.functions:
        for blk in f.blocks:
            new = []
            for inst in blk.instructions:
                si = inst.sync_info
                if si is not None and len(si.on_wait) > 1:
                    waits = list(si.on_wait)
                    for w in waits[:-1]:
                        cnt += 1
                        n = mybir.InstNoOp(name=f"I-dsplit-{cnt}", ins=[], outs=[])
                        n.engine = inst.engine
                        n.sync_info = mybir.SyncInfo(on_wait=[w], on_update=[])
                        new.append(n)
                    inst.sync_info = mybir.SyncInfo(
                        on_wait=[waits[-1]], on_update=list(si.on_update)
                    )
                new.append(inst)
            blk.instructions = new
    return cnt


def _build_nc():
    nc = bass.Bass()

    # DRAM parameters (host-prepped layouts)
    d_st0 = nc.dram_tensor("st0", [128, NT, N], F8, kind="ExternalInput")
    d_st1 = nc.dram_tensor("st1", [128, NT, NT, 128], F8, kind="ExternalInput")
    d_x0 = nc.dram_tensor("x0h", [128, NT, BL, C], F16, kind="ExternalInput")
    d_x08 = nc.dram_tensor("x08", [128, NT, BL, C], F8, kind="ExternalInput")
    d_x0ti = nc.dram_tensor("x0ti", [3, BL, N], F16, kind="ExternalInput")
    d_wru = nc.dram_tensor("wru", [C + 1, M, 2 * U], F16, kind="ExternalInput")
    d_wc = nc.dram_tensor("wc", [C + 1, M, U], F16, kind="ExternalInput")
    d_out = nc.dram_tensor("out", [NT, 128, BL, U], F16, kind="ExternalOutput")

    with tile.TileContext(nc) as tc:
        with (
            tc.tile_pool(name="const", bufs=1) as const_pool,
            tc.tile_pool(name="xbufs", bufs=1) as xbufs,
            tc.tile_pool(name="xtq", bufs=3) as xtq_pool,
            tc.tile_pool(name="cbuf", bufs=2) as c_pool,
            tc.tile_pool(name="pre", bufs=2) as pre_pool,
            tc.tile_pool(name="rp", bufs=2) as r_pool,
            tc.tile_pool(name="dps", bufs=3, space="PSUM") as diff_ps,
            tc.tile_pool(name="tps", bufs=3, space="PSUM") as tr_ps,
            tc.tile_pool(name="nps", bufs=2, space="PSUM") as dense_ps,
        ):
            # ---- resident constants ----
            # All DRAM layouts are partition-major (contiguous 2KB+ lines per
            # partition).  Descriptor issue costs ~1.3us of the issuing
            # engine, and a queue's descriptors transfer mostly serially, so
            # use ~350-700KB descriptors interleaved across the three
            # DMA-capable queues (sync/gpsimd/scalar; scalar is idle early)
            # in consumption order: x08+st0 gate the first matmul, x0 is
            # needed at hop-2 (~+20us), st1 at pass 3 (~+35us, consumed in
            # nt order), x0t at dense-0, hxf at dense-1.
            x08 = xbufs.tile([128, NT, BL, C], F8, tag="x08")
            st0 = const_pool.tile([128, NT, N], F8)
            st1 = const_pool.tile([128, NT, NT, 128], F8)
            x0 = xbufs.tile([128, NT, BL, C], F16, tag="x0")
            x0t = xbufs.tile([C + 1, BL, N], F16, tag="x0t")
            # Observed queue rates: scalar ~120GB/s, sync ~55, gpsimd ~25-50.
            # Split every tensor across queues proportional to speed, ordered
            # by consumption deadline: x08+st0 (first matmul), x0 (hop-2),
            # x0t+w (half-dense-A), st1 (S1 passes, consumed in nt order).
            engs3 = [nc.scalar, nc.sync, nc.gpsimd]
            shares = [(0, 6), (6, 12), (12, 16)]
            for eng, (lo, hi) in zip(engs3, shares):
                eng.dma_start(out=x08[:, lo:hi], in_=d_x08[:, lo:hi])
            for eng, (lo, hi) in zip(engs3, shares):
                mid = (lo + hi) // 2
                eng.dma_start(out=st0[:, lo:mid, :], in_=d_st0[:, lo:mid, :])
                eng.dma_start(out=st0[:, mid:hi, :], in_=d_st0[:, mid:hi, :])
            for eng, (lo, hi) in zip(engs3, shares):
                eng.dma_start(out=x0[:, lo:hi], in_=d_x0[:, lo:hi])
            ident = const_pool.tile([128, 128], F16)
            make_identity(nc, ident)
            wall = const_pool.tile([C + 1, M, 3 * U], F16)
            wru = wall[:, :, 0:2 * U]
            wc = wall[:, :, 2 * U:3 * U]
            nc.sync.dma_start(out=wru, in_=d_wru[:, :, :])
            nc.sync.dma_start(out=wc, in_=d_wc[:, :, :])
            nc.scalar.dma_start(out=x0t[U:C + 1], in_=d_x0ti[:])
            # st1 pieces in nt (consumption) order, round-robin by speed
            st1_engs = [nc.scalar, nc.sync, nc.gpsimd]
            for i in range(8):
                eng = st1_engs[i % 3]
                eng.dma_start(out=st1[:, 2 * i:2 * i + 2],
                              in_=d_st1[:, 2 * i:2 * i + 2])

            # diffusion outputs (m=1..4), full 66 channels, fp16
            xh = [xbufs.tile([128, NT, BL, C], F16, tag=f"xh{i}", name=f"xh{i}")
                  for i in range(4)]
            # fp8 copies of hop-1 results (rhs of hop-2), one per support
            x18 = [xbufs.tile([128, NT, BL, C], F8, tag=f"x18_{s}", name=f"x18_{s}")
                   for s in range(2)]
            x18g1 = [xbufs.tile([128, NT, BL, U], F8, tag=f"x18g1_{s}",
                                name=f"x18g1_{s}") for s in range(2)]
            # gconv2 state r*hx (u-columns only)
            xhp = xbufs.tile([128, NT, BL, U], F16, tag="xhp")
            xhp8 = xbufs.tile([128, NT, BL, U], F8, tag="xhp8")
            u_sb = xbufs.tile([128, NT, BL, U], F16, tag="u")
            # gconv1 partial dense accumulator (m in {0,1,2})
            pd = xbufs.tile([128, NT, BL, 2 * U], F16, tag="pd")

            def diffusion_pass(gi, s, hop):
                """One S-application; writes xh[2s+hop] (u-cols when gi=1)."""
                nfree = C if gi == 0 else U
                x_first8 = x08 if gi == 0 else xhp8
                dst = xh[2 * s + hop]
                d8buf = x18[s] if gi == 0 else x18g1[s]
                for nt in range(NT):
                    ps = diff_ps.tile([128, BL, nfree], F32, tag="dps")
                    for mc in range(0, NT, 2):
                        if s == 0:
                            lhsT = st0[:, mc:mc + 2, nt * 128:(nt + 1) * 128]
                        else:
                            lhsT = st1[:, nt, mc:mc + 2, :]
                        rhs = x_first8[:, mc:mc + 2] if hop == 0 else d8buf[:, mc:mc + 2]
                        nc.tensor.matmul(
                            ps, lhsT, rhs,
                            start=(mc == 0), stop=(mc == NT - 2),
                            perf_mode=DR,
                        )
                    if gi == 0:
                        dst_ap = dst[:, nt, :, :]
                        sub = x0[:, nt, :, :]
                    else:
                        dst_ap = dst[:, nt, :, 0:U]
                        sub = xhp[:, nt, :, :]
                    if hop == 0:
                        # xh = 2*y1 (stored scaled; W rows halved on host)
                        nc.vector.tensor_scalar_mul(dst_ap, ps, 2.0 / SS)
                        # fp8 copy = SX1*y1 for the hop-2 S-apply
                        nc.scalar.activation(
                            out=d8buf[:, nt],
                            in_=ps,
                            func=mybir.ActivationFunctionType.Copy,
                            scale=SX1 / SS,
                        )
                    else:
                        # x2 = 2*(S y1) - x0
                        nc.vector.scalar_tensor_tensor(
                            out=dst_ap,
                            in0=ps,
                            scalar=2.0 / (SS * SX1),
                            in1=sub,
                            op0=mybir.AluOpType.mult,
                            op1=mybir.AluOpType.subtract,
                        )

            def stage_transposes(gi, q, ms):
                """PE transposes + DVE/ScalarE copies for this chunk's xT."""
                nt = q
                qs = slice(nt * 128, (nt + 1) * 128)
                xtc = xtq_pool.tile([C + 1, M, BL, 128], F16, tag="xt",
                                    name=f"xt_g{gi}_q{q}_{ms[0]}")
                if gi == 1:
                    # m=0 tile: xhp^T (PE transpose) + inp/ones rows from the
                    # resident x0t (small SBUF-SBUF copy)
                    nc.vector.tensor_copy(xtc[U:C + 1, 0, :, :], x0t[U:C + 1, :, qs])
                    pst = tr_ps.tile([C, BL, 128], F16, tag="tps", name="pst0")
                    for b in range(BL):
                        nc.tensor.transpose(pst[0:U, b, :], xhp[:, nt, b, :], ident)
                    nc.vector.tensor_copy(xtc[0:U, 0, :, :], pst[0:U])
                if gi == 0 and ms[0] == 1:
                    # build x0t u-rows (hx^T) for this chunk on-device
                    pstx = tr_ps.tile([C, BL, 128], F16, tag="tps", name="pstx")
                    for b in range(BL):
                        nc.tensor.transpose(
                            pstx[0:U, b, :], x0[:, nt, b, 0:U], ident
                        )
                    nc.scalar.activation(
                        out=x0t[0:U, :, qs],
                        in_=pstx[0:U],
                        func=mybir.ActivationFunctionType.Copy,
                    )
                for m in ms:
                    srcb = xh[m - 1]
                    pst = tr_ps.tile([C, BL, 128], F16, tag="tps", name="pstm")
                    for b in range(BL):
                        nc.tensor.transpose(pst[:, b, :], srcb[:, nt, b, :], ident)
                    # split PSUM->SBUF copies between DVE and ScalarE so one
                    # engine's copy backlog does not stall the PE
                    if m % 2 == 1:
                        nc.vector.tensor_copy(xtc[0:C, m, :, :], pst)
                    else:
                        nc.scalar.activation(
                            out=xtc[0:C, m, :, :],
                            in_=pst,
                            func=mybir.ActivationFunctionType.Copy,
                        )
                return xtc

            def dense_half_a(q, xtc):
                """gconv1 partial: m in {0,1,2} (I + S0 chain) -> pd (f16)."""
                nt = q
                qs = slice(nt * 128, (nt + 1) * 128)
                dps = dense_ps.tile([128, BL, 2 * U], F32, tag="nps")
                for b in range(BL):
                    for m in (0, 1, 2):
                        k = C + 1 if m == 0 else C
                        lhsT = x0t[0:k, b, qs] if m == 0 else xtc[0:k, m, b, :]
                        nc.tensor.matmul(
                            dps[:, b, :], lhsT, wru[0:k, m, :],
                            start=(m == 0), stop=(m == 2),
                        )
                nc.vector.tensor_copy(pd[:, nt], dps)

            def dense_half_b(q, xtc):
                """gconv1 rest: m in {3,4} + pd -> sigmoids, xhp, xhp8."""
                nt = q
                dps = dense_ps.tile([128, BL, 2 * U], F32, tag="nps")
                for b in range(BL):
                    for m in (3, 4):
                        nc.tensor.matmul(
                            dps[:, b, :], xtc[0:C, m, b, :], wru[0:C, m, :],
                            start=(m == 3), stop=(m == 4),
                        )
                pre = pre_pool.tile([128, BL, 2 * U], F16, tag="pre")
                nc.vector.scalar_tensor_tensor(
                    out=pre, in0=dps, scalar=1.0, in1=pd[:, nt],
                    op0=mybir.AluOpType.mult, op1=mybir.AluOpType.add,
                )
                rt = r_pool.tile([128, BL, U], F16, tag="rt")
                nc.scalar.activation(
                    out=rt, in_=pre[:, :, 0:U],
                    func=mybir.ActivationFunctionType.Sigmoid,
                )
                nc.scalar.activation(
                    out=u_sb[:, nt, :, :], in_=pre[:, :, U:2 * U],
                    func=mybir.ActivationFunctionType.Sigmoid,
                )
                nc.vector.tensor_mul(xhp[:, nt, :, :], rt, x0[:, nt, :, 0:U])
                nc.scalar.activation(
                    out=xhp8[:, nt], in_=xhp[:, nt],
                    func=mybir.ActivationFunctionType.Copy, scale=1.0,
                )

            def stage_dense1(q, xtc):
                """gconv2 full dense + output combine."""
                nt = q
                dps = dense_ps.tile([128, BL, U], F32, tag="nps")
                for b in range(BL):
                    for m in range(M):
                        k = C + 1 if m == 0 else C
                        nc.tensor.matmul(
                            dps[:, b, :], xtc[0:k, m, b, :], wc[0:k, m, :],
                            start=(m == 0), stop=(m == M - 1),
                        )
                cpair = c_pool.tile([128, 2, BL, U], F16, tag="cb")
                cb = cpair[:, 0]
                tmp = cpair[:, 1]
                nc.scalar.activation(
                    out=cb, in_=dps, func=mybir.ActivationFunctionType.Tanh
                )
                # new = c + u*(hx - c); hx is the u-column block of x0
                nc.vector.tensor_sub(tmp, x0[:, nt, :, 0:U], cb)
                nc.vector.tensor_mul(tmp, u_sb[:, nt, :, :], tmp)
                nc.vector.tensor_add(tmp, tmp, cb)
                oeng = [nc.sync, nc.scalar, nc.gpsimd][nt % 3]
                oeng.dma_start(out=d_out[nt], in_=tmp)

            def pipeline(ms, dense_fn, gi):
                prev = stage_transposes(gi, 0, ms)
                for q in range(1, NQ):
                    cur = stage_transposes(gi, q, ms)
                    dense_fn(q - 1, prev)
                    prev = cur
                dense_fn(NQ - 1, prev)

            # gconv1: S0 passes, then the S0-half of the dense stage (covers
            # the st1 DMA), then S1 passes and the rest of the dense stage
            diffusion_pass(0, 0, 0)
            diffusion_pass(0, 0, 1)
            pipeline((1, 2), dense_half_a, 0)
            diffusion_pass(0, 1, 0)
            diffusion_pass(0, 1, 1)
            pipeline((3, 4), dense_half_b, 0)
            # gconv2
            diffusion_pass(1, 0, 0)
            diffusion_pass(1, 0, 1)
            diffusion_pass(1, 1, 0)
            diffusion_pass(1, 1, 1)
            pipeline((1, 2, 3, 4), stage_dense1, 1)

    _split_drain_waits(nc)
    return nc


_NC_CACHE = None


def _get_nc():
    global _NC_CACHE
    if _NC_CACHE is None:
        _NC_CACHE = _build_nc()
    return _NC_CACHE


def _prep_host(inputs, hx, support0, support1, W_ru, b_ru, W_c, b_c):
    f16 = np.float16
    f8 = ml_dtypes.float8_e4m3
    inp = inputs.reshape(B, N, D).astype(np.float32)
    hx3 = hx.reshape(B, N, U).astype(np.float32)
    x0_full = np.concatenate([hx3, inp], axis=2)  # [B, N, C] fp32, u-first

    # partition-major: st0[p, mc, col] = S0^T[mc*128+p, col]
    st0 = np.ascontiguousarray(
        (support0.T.reshape(NT, 128, N).transpose(1, 0, 2)) * SS
    ).astype(f8)
    # st1[p, nt, mc, col] = S1^T[mc*128+p, nt*128+col]
    st1 = np.ascontiguousarray(
        (support1.T.reshape(NT, 128, NT, 128).transpose(1, 2, 0, 3)) * SS
    ).astype(f8)

    def prep_w(W, bvec, osz):
        w = W.reshape(C, M, osz).astype(np.float32)
        w = np.concatenate([w[D:], w[:D]], axis=0).copy()  # u-first rows
        w[:, 1, :] *= 0.5
        w[:, 3, :] *= 0.5
        wf = np.zeros((C + 1, M, osz), np.float32)
        wf[:C] = w
        wf[C, 0, :] = bvec
        return wf.astype(f16)

    wru = prep_w(W_ru, b_ru, 2 * U)
    wcc = prep_w(W_c, b_c, U)

    in_maps = []
    for c in range(NCORES):
        cs = slice(c * BL, (c + 1) * BL)
        x0c = x0_full[cs]                                   # [BL, N, C]
        x0h32 = np.ascontiguousarray(
            x0c.transpose(1, 0, 2).reshape(NT, 128, BL, C).transpose(1, 0, 2, 3)
        )                                                    # [128, NT, BL, C]
        x0ti = np.concatenate(
            [x0c.transpose(2, 0, 1)[U:], np.ones((1, BL, N), np.float32)], axis=0
        ).astype(f16)                                        # [3, BL, N]
        in_maps.append(
            {
                "st0": st0,
                "st1": st1,
                "x0h": np.ascontiguousarray(x0h32.astype(f16)),
                "x08": np.ascontiguousarray(x0h32.astype(f8)),
                "x0ti": np.ascontiguousarray(x0ti),
                "wru": wru,
                "wc": wcc,
            }
        )
    return in_maps


def kernel(inputs, hx, support0, support1, W_ru, b_ru, W_c, b_c, _trace=False,
           _tmpdir=None):
    nc = _get_nc()
    in_maps = _prep_host(
        inputs, hx, support0, support1, W_ru, b_ru, W_c, b_c
    )
    res = run_bass_kernel_spmd(
        nc, in_maps, core_ids=list(range(NCORES)), trace=_trace, tmpdir=_tmpdir
    )
    out = np.empty((B, N * U), np.float32)
    for c in range(NCORES):
        od = res.results[c]["out"].astype(np.float32)  # [NT, 128, BL, U] f16
        out[c * BL:(c + 1) * BL] = (
            od.transpose(2, 0, 1, 3).reshape(BL, N * U)
        )
    kernel._last_result = res
    return out
